# revision 1
# baseline (speedup 1.0000x reference)
"""Bass/Tile GRU kernel for trn2, data-parallel over batch on 8 cores.

Model: xe = emb[x]; gi = xe @ w_ih.T + b_ih; per step:
  gh = h @ w_hh.T (+ b_hh)
  r = sig(gi_r + gh_r + bhh_r); z = sig(gi_z + gh_z + bhh_z)
  n = tanh(gi_n + r * (gh_n + bhh_n)); h = (1-z)*n + z*h
then logits = outs @ fc_w.T + fc_b; out = log_softmax(logits)

Per-core layout (B_loc = 8 batch rows per core):
  - "strip" s in 0..3 owns hidden dims 256s..256s+255. Within-strip gate
    column order is [r(256) | z(256) | n(256)] -> 768 cols per strip.
  - gate tensors live as [128, 768]: partition = 32*s + 8*rep + b
    (batch replicated 4x so all 128 partitions are valid), free = in-strip col.
  - h (batch-major) is [128, 256]: partition as above, free = hidden-in-strip.
  - hT (h-major, the matmul lhsT + the stored output) is [128, 256]:
    partition = hidden-in-k-tile, free = 32*k_tile + 8*rep + b.
Matmuls for gh use 4-way column tiling: psum rows 32s..32s+31 per strip,
k-tiles of 128 over hidden, rhs = pre-shuffled w_hh.T slices.
"""

import numpy as np
from contextlib import ExitStack

import concourse.bass as bass
import concourse.tile as tile
from concourse import bacc, mybir

F32 = mybir.dt.float32
BF16 = mybir.dt.bfloat16
I16 = mybir.dt.int16
AF = mybir.ActivationFunctionType

VOCAB, D_IN, D_H, D_OUT, B, S = 32000, 512, 1024, 64, 64, 256
P = 128
NCORES = 8
BL = B // NCORES          # 8 batch rows per core
NT = BL * S               # 2048 (b,t) rows per core
FLUSH = 32                # steps between outsT flushes


def gate_cols():
    """Map strip-ordered column index c (0..3071) -> original gate column."""
    c = np.arange(3 * D_H)
    s = c // 768
    j = c % 768
    g = j // 256
    jj = j % 256
    return g * D_H + s * 256 + jj  # g=0 r, 1 z, 2 n


def host_prep(x, emb, w_ih, w_hh, b_ih, b_hh, fc_w, fc_b):
    """Produce the per-core and shared input arrays for the bass kernel."""
    gcol = gate_cols()

    import ml_dtypes
    emb_bf = emb.astype(ml_dtypes.bfloat16)
    # w_ih: [3H, D_IN]; wih_rhs[k, p, c] = w_ih[gcol[c], 128k+p]
    wih_rhs = np.ascontiguousarray(
        w_ih[gcol, :].T.reshape(4, P, 3 * D_H)
    ).astype(ml_dtypes.bfloat16)
    whh_rhs = np.ascontiguousarray(
        w_hh[gcol, :].T.reshape(8, P, 3 * D_H)
    ).astype(ml_dtypes.bfloat16)
    g = gcol // D_H
    bgi = (b_ih[gcol] + np.where(g < 2, b_hh[gcol], 0.0)).astype(np.float32)  # [3072]
    bhn = b_hh[2 * D_H:].reshape(4, 256).astype(np.float32)  # [s, jj]
    fcw_k = fc_w.T.reshape(8, P, D_OUT)
    khs = np.arange(8)
    fcw_l = np.ascontiguousarray(
        fcw_k[2 * (khs % 4) + khs // 4]).astype(ml_dtypes.bfloat16)
    fcb = fc_b.astype(np.float32).reshape(D_OUT, 1)
    iden16 = np.eye(P, dtype=np.float32).astype(ml_dtypes.bfloat16)
    pcol = np.arange(P)[:, None]
    repsel = ((pcol < 8) & (np.arange(32)[None, :] % 8 == pcol)).astype(
        np.float32).astype(ml_dtypes.bfloat16)  # [128, 32] replication selector
    iden32 = np.eye(P, dtype=np.float32)

    shared = dict(
        emb=emb_bf, wih=wih_rhs, whh=whh_rhs, bgi=bgi, bhn=bhn,
        fcw=fcw_l, fcb=fcb, iden16=iden16, iden32=iden32, repsel=repsel,
    )
    per_core = []
    for c in range(NCORES):
        ids = np.ascontiguousarray(
            np.asarray(x[c * BL:(c + 1) * BL, :S]).T).reshape(-1).astype(np.int16)
        tmp = np.zeros((16, P), np.int16)
        i = np.arange(NT)
        li = i % 512
        tmp[li % 16, (i // 512) * 32 + li // 16] = ids
        idx = np.tile(tmp, (8, 1))  # replicated for the 8 Q7 cores
        per_core.append({"idx": idx, **shared})
    return per_core


def build_kernel(s_steps=S, gates_dt=BF16, stage=3):
    """Build the bacc program. stage: 0=setup only, 1=+phase1, 2=+recurrence, 3=full."""
    import os as _os
    stage = int(_os.environ.get("GATE_STAGE", str(stage)))
    nc = bacc.Bacc("TRN2", debug=False, num_devices=1)

    idx = nc.dram_tensor("idx", [P, P], I16, kind="ExternalInput").ap()
    emb = nc.dram_tensor("emb", [VOCAB, D_IN], BF16, kind="ExternalInput").ap()
    wih = nc.dram_tensor("wih", [4, P, 3 * D_H], BF16, kind="ExternalInput").ap()
    whh = nc.dram_tensor("whh", [8, P, 3 * D_H], BF16, kind="ExternalInput").ap()
    bgi = nc.dram_tensor("bgi", [3 * D_H], F32, kind="ExternalInput").ap()
    bhn = nc.dram_tensor("bhn", [4, 256], F32, kind="ExternalInput").ap()
    fcw = nc.dram_tensor("fcw", [8, P, D_OUT], BF16, kind="ExternalInput").ap()
    fcb = nc.dram_tensor("fcb", [D_OUT, 1], F32, kind="ExternalInput").ap()
    iden16 = nc.dram_tensor("iden16", [P, P], BF16, kind="ExternalInput").ap()
    repsel = nc.dram_tensor("repsel", [P, 32], BF16, kind="ExternalInput").ap()
    iden32 = nc.dram_tensor("iden32", [P, P], F32, kind="ExternalInput").ap()
    out = nc.dram_tensor("out", [NT, D_OUT], F32, kind="ExternalOutput").ap()

    n_mtiles = NT // P  # 16 (only s_steps*BL/128 used when s_steps < S)
    n_used_mtiles = NT // P  # all (b,t) tiles; rows are b-major

    with tile.TileContext(nc) as tc, ExitStack() as ctx:
        singles = ctx.enter_context(tc.tile_pool(name="singles", bufs=1))
        dram = ctx.enter_context(tc.tile_pool(name="dram", bufs=1, space="DRAM"))

        # ---- persistent SBUF state ----
        whh_k = [singles.tile([P, 3 * D_H], BF16, tag=f"whh{k}", name=f"whh_k{k}")
                 for k in range(8)]
        for k in range(8):
            nc.sync.dma_start(whh_k[k][:], whh[k])
        wih_k = [singles.tile([P, 3 * D_H], BF16, tag=f"wih{k}", name=f"wih_k{k}")
                 for k in range(4)]
        for k in range(4):
            nc.sync.dma_start(wih_k[k][:], wih[k])
        bgi_sb = singles.tile([P, 3 * D_H], F32)
        nc.sync.dma_start(
            bgi_sb[:],
            bass.AP(tensor=bgi.tensor, offset=bgi.offset, ap=[[0, P], [1, 3 * D_H]]),
        )
        bhn_sb = singles.tile([P, 256], F32)
        nc.sync.dma_start(
            bhn_sb[:],
            bass.AP(tensor=bhn.tensor, offset=bhn.offset,
                    ap=[[256, 4], [0, 32], [1, 256]]),
        )
        fcw_sb = singles.tile([P, 8, D_OUT], BF16)
        nc.sync.dma_start(fcw_sb[:], fcw.rearrange("k p c -> p k c"))
        fcb_sb = singles.tile([D_OUT, 1], F32)
        nc.sync.dma_start(fcb_sb[:], fcb)
        iden16_sb = singles.tile([P, P], BF16)
        nc.sync.dma_start(iden16_sb[:], iden16)
        repsel_sb = singles.tile([P, 32], BF16)
        nc.sync.dma_start(repsel_sb[:], repsel)
        iden32_sb = singles.tile([P, P], F32)
        nc.sync.dma_start(iden32_sb[:], iden32)
        idx_sb = singles.tile([P, P], I16)
        nc.sync.dma_start(idx_sb[:], idx)

        # gathered embeddings, already transposed: xeT[p, kc, i] = xe[i, 128*kc+p]
        import os as _os2
        xeT = singles.tile([P, NT // 512, 4, 512], BF16)
        if _os2.environ.get("GATE_GATHER", "1") == "0":
            nc.vector.memset(xeT[:], 0.0)
        else:
            for gc in range(NT // 512):
                nc.gpsimd.dma_gather(
                    out_ap=xeT[:, gc],
                    in_ap=emb,
                    idxs_ap=idx_sb[:, gc * 32:(gc + 1) * 32],
                    num_idxs=512,
                    num_idxs_reg=512,
                    elem_size=D_IN,
                    transpose=True,
                )

        # DRAM scratch
        gi_dram = dram.tile([S, 4, BL, 768], gates_dt)
        outsT = dram.tile([8, P, NT], BF16)

        # ---- phase 1: gi = xe @ w_ih.T + bias, strip-ordered.
        # Emitted per M-tile, interleaved with the recurrence so its matmuls
        # fill the PE gaps left by the per-step gate chain (keeps the PE
        # p-state warm and hides phase 1 entirely).
        p1 = ctx.enter_context(tc.tile_pool(name="p1", bufs=3))
        p1psum = ctx.enter_context(tc.tile_pool(name="p1psum", bufs=1, space="PSUM"))

        def emit_p1_tile(mi):
            t0 = 16 * mi
            for cc in range(8):  # 384-col chunks
                ps = p1psum.tile([P, 384], F32, tag="p1ps", name=f"p1ps_{mi}_{cc}")
                for k in range(4):
                    nc.tensor.matmul(
                        ps[:],
                        lhsT=xeT[:, mi // 4, k,
                                 128 * (mi % 4):128 * (mi % 4) + 128],
                        rhs=wih_k[k][:, cc * 384:(cc + 1) * 384],
                        start=(k == 0), stop=(k == 3),
                    )
                gi_sb = p1.tile([P, 384], gates_dt, tag="p1gi", name=f"p1gi_{mi}_{cc}")
                nc.vector.tensor_add(gi_sb[:], ps[:], bgi_sb[:, cc * 384:(cc + 1) * 384])
                nc.sync.dma_start(
                    gi_dram[t0:t0 + 16, cc // 2, :,
                            (cc % 2) * 384:(cc % 2) * 384 + 384],
                    gi_sb[:],
                )

        if stage >= 1:
            emit_p1_tile(0)

        # ---- phase 2: recurrence ----
        hbufs = ctx.enter_context(tc.tile_pool(name="hbufs", bufs=3))
        gi_pool = ctx.enter_context(tc.tile_pool(name="gi", bufs=4))
        gpool = ctx.enter_context(tc.tile_pool(name="gates", bufs=3))
        flushp = ctx.enter_context(tc.tile_pool(name="flush", bufs=2))
        p2ctx = ExitStack()
        psg = p2ctx.enter_context(tc.tile_pool(name="psg", bufs=2, space="PSUM"))
        ging = p2ctx.enter_context(tc.tile_pool(name="ging", bufs=1, space="PSUM"))
        pst = p2ctx.enter_context(tc.tile_pool(name="pst", bufs=1, space="PSUM"))

        h_prev = [singles.tile([P, 128], gates_dt, tag=f"h0_{hh}", name=f"h_prev{hh}") for hh in range(2)]
        hT_prev = [singles.tile([P, 64], BF16, tag=f"hT0_{q}", name=f"hT_prev{q}") for q in range(4)]
        for hh in range(2):
            nc.vector.memset(h_prev[hh][:], 0.0)
        for q in range(4):
            nc.vector.memset(hT_prev[q][:], 0.0)

        flush_per = min(FLUSH, s_steps)
        flush_buf = None
        for t in range(s_steps if stage >= 2 else 0):
            if t % 16 == 0 and t // 16 + 1 < n_used_mtiles and stage >= 1:
                emit_p1_tile(t // 16 + 1)
            tl = t % flush_per
            if tl == 0:
                flush_buf = flushp.tile([P, 8, flush_per, BL], BF16)

            gi4 = gi_pool.tile([BL, 4, 768], gates_dt, tag="gi4")
            gi_src = gi_dram[t]  # [4, BL, 768] -> read as (b, s, j)
            nc.sync.dma_start(
                gi4[:],
                bass.AP(tensor=gi_src.tensor, offset=gi_src.offset,
                        ap=[list(gi_src.ap[1]), list(gi_src.ap[0]),
                            list(gi_src.ap[2])]),
            )

            # gh matmuls: 4 column strips, accumulate over 8 k-tiles,
            # then fold in gi_r/gi_z (replicated across reps) via repsel.
            psum_g = psg.tile([P, 768], F32)
            gin_ps = ging.tile([P, 256], F32)
            for k in (0, 2, 4, 6, 1, 3, 5, 7):
                q = 2 * (k % 2) + (k // 2) // 2
                lhsT = hT_prev[q][:, 32 * ((k // 2) % 2):32 * ((k // 2) % 2) + 32]
                for s in range(4):
                    ps = psum_g[32 * s:32 * s + 32, :]
                    nc.tensor.matmul(
                        ps[:, 0:512], lhsT=lhsT,
                        rhs=whh_k[k][:, 768 * s:768 * s + 512],
                        start=(k == 0), stop=False,
                        tile_position=(0, 32 * s),
                        skip_group_check=True,
                    )
                    nc.tensor.matmul(
                        ps[:, 512:768], lhsT=lhsT,
                        rhs=whh_k[k][:, 768 * s + 512:768 * s + 768],
                        start=(k == 0), stop=(k == 7),
                        tile_position=(0, 32 * s),
                        skip_group_check=True,
                    )
            for s in range(4):
                ps = psum_g[32 * s:32 * s + 32, :]
                nc.tensor.matmul(
                    ps[:, 0:512], lhsT=repsel_sb[0:8, :],
                    rhs=gi4[0:8, s, 0:512],
                    start=False, stop=True,
                    tile_position=(0, 32 * s),
                    skip_group_check=True,
                )
                nc.tensor.matmul(
                    gin_ps[32 * s:32 * s + 32, :],
                    lhsT=repsel_sb[0:8, :],
                    rhs=gi4[0:8, s, 512:768],
                    start=True, stop=True,
                    tile_position=(0, 32 * s),
                    skip_group_check=True,
                )

            # gates: n-path first; z sigmoid deferred past tanh so ACT
            # serves the critical chain (r -> n) with minimum latency.
            t_n = gpool.tile([P, 256], gates_dt, tag="t_n")
            nc.vector.tensor_add(t_n[:], psum_g[:, 512:768], bhn_sb[:])
            r_g = gpool.tile([P, 256], gates_dt, tag="r_g")
            nc.scalar.activation(r_g[:], psum_g[:, 0:256], AF.Sigmoid)
            t_n2 = gpool.tile([P, 256], gates_dt, tag="t_n2")
            nc.vector.tensor_mul(t_n2[:], r_g[:], t_n[:])
            t_n3 = gpool.tile([P, 256], gates_dt, tag="t_n3")
            nc.vector.tensor_add(t_n3[:], t_n2[:], gin_ps[:])
            n_g = [gpool.tile([P, 128], gates_dt, tag=f"n_g{hh}", name=f"n_gt{hh}")
                   for hh in range(2)]
            z_g = [gpool.tile([P, 128], gates_dt, tag=f"z_g{hh}", name=f"z_gt{hh}")
                   for hh in range(2)]
            for hh in range(2):
                nc.scalar.activation(n_g[hh][:],
                                     t_n3[:, 128 * hh:128 * hh + 128], AF.Tanh)
                nc.scalar.activation(z_g[hh][:],
                                     psum_g[:, 256 + 128 * hh:384 + 128 * hh],
                                     AF.Sigmoid)

            # h update + transpose, fully split by halves (separate tiles so
            # Tile's RAW tracking lets each transpose/copy/MM start per-half).
            h_new = [hbufs.tile([P, 128], gates_dt, tag=f"h{hh}", name=f"h_new{hh}") for hh in range(2)]
            psum_T = [pst.tile([P, 128], BF16, tag=f"pT{hh}", name=f"psum_T{hh}") for hh in range(2)]
            hT_new = [hbufs.tile([P, 64], BF16, tag=f"hTq{q}", name=f"hT_q{q}")
                      for q in range(4)]
            for hh in range(2):
                d_g = gpool.tile([P, 128], gates_dt, tag=f"d_g{hh}")
                nc.vector.tensor_sub(d_g[:], h_prev[hh][:], n_g[hh][:])
                t5 = gpool.tile([P, 128], gates_dt, tag=f"t5{hh}")
                nc.vector.tensor_mul(t5[:], z_g[hh][:], d_g[:])
                nc.vector.tensor_add(h_new[hh][:], n_g[hh][:], t5[:])
                nc.tensor.transpose(psum_T[hh][:], h_new[hh][:], iden16_sb[:])
                nc.vector.tensor_copy(hT_new[2 * hh][:], psum_T[hh][:, 0:64])
                nc.vector.tensor_copy(hT_new[2 * hh + 1][:], psum_T[hh][:, 64:128])
            for hh in range(2):
                pv = psum_T[hh].rearrange("p (s rb) -> p s rb", rb=32)
                nc.vector.tensor_copy(
                    flush_buf[:, 4 * hh:4 * hh + 4, tl, :], pv[:, :, 0:BL])

            if tl == flush_per - 1:
                tc0 = (t // flush_per) * flush_per * BL
                nc.sync.dma_start(
                    outsT.rearrange("k p n -> p k n")[:, :, tc0:tc0 + flush_per * BL],
                    flush_buf[:],
                )

            h_prev = h_new
            hT_prev = hT_new

        if stage >= 1:
            for mi in range(max(1, (s_steps - 1) // 16 + 2) if stage >= 2 else 1,
                            n_used_mtiles):
                emit_p1_tile(mi)

        p2ctx.close()

        # ---- phase 3: fc + log_softmax ----
        n_rows = s_steps * BL
        with tc.tile_pool(name="p3", bufs=3) as p3, \
             tc.tile_pool(name="p3psum", bufs=2, space="PSUM") as p3psum, \
             tc.tile_pool(name="p3psum2", bufs=2, space="PSUM") as p3psum2:
            for q in range((n_rows + 511) // 512 if stage >= 3 else 0):
                cw = min(512, n_rows - q * 512)
                oT = p3.tile([P, 8, 512], BF16, tag="oT")
                nc.sync.dma_start(
                    oT[:, :, :cw],
                    outsT.rearrange("k p n -> p k n")[:, :, q * 512:q * 512 + cw],
                )
                ps3 = p3psum.tile([D_OUT, 512], F32)
                for k in range(8):
                    nc.tensor.matmul(
                        ps3[:, :cw], lhsT=fcw_sb[:, k, :], rhs=oT[:, k, :cw],
                        start=(k == 0), stop=(k == 7),
                    )
                logitsT = p3.tile([D_OUT, 512], F32, tag="logitsT")
                nc.vector.tensor_scalar_add(logitsT[:, :cw], ps3[:, :cw], fcb_sb[:])
                for w in range((cw + 127) // 128):
                    pw = min(128, cw - w * 128)
                    ps4 = p3psum2.tile([P, D_OUT], F32)
                    nc.tensor.transpose(
                        ps4[:pw, :], logitsT[:, w * 128:w * 128 + pw],
                        iden32_sb[0:D_OUT, 0:D_OUT],
                    )
                    lg = p3.tile([P, D_OUT], F32, tag="lg")
                    nc.vector.tensor_copy(lg[:pw], ps4[:pw])
                    mx = p3.tile([P, 1], F32, tag="mx")
                    nc.vector.reduce_max(mx[:pw], lg[:pw], axis=mybir.AxisListType.X)
                    nmx = p3.tile([P, 1], F32, tag="nmx")
                    nc.scalar.mul(nmx[:pw], mx[:pw], -1.0)
                    ex = p3.tile([P, D_OUT], F32, tag="ex")
                    nc.scalar.activation(ex[:pw], lg[:pw], AF.Exp, bias=nmx[:pw])
                    sm = p3.tile([P, 1], F32, tag="sm")
                    nc.vector.reduce_sum(sm[:pw], ex[:pw], axis=mybir.AxisListType.X)
                    lsm = p3.tile([P, 1], F32, tag="lsm")
                    nc.scalar.activation(lsm[:pw], sm[:pw], AF.Ln)
                    off = p3.tile([P, 1], F32, tag="off")
                    nc.vector.tensor_add(off[:pw], mx[:pw], lsm[:pw])
                    res = p3.tile([P, D_OUT], F32, tag="res")
                    nc.vector.tensor_scalar_sub(res[:pw], lg[:pw], off[:pw])
                    nc.sync.dma_start(
                        out[q * 512 + w * 128:q * 512 + w * 128 + pw, :], res[:pw]
                    )

        if stage < 3:
            with tc.tile_pool(name="dbg", bufs=1) as dbg:
                z = dbg.tile([P, D_OUT], F32)
                nc.vector.memset(z[:], 1.0)
                nc.sync.dma_start(out[0:P, :], z[:])

    nc.compile()
    return nc


def assemble_output(core_outs, s_steps=S):
    """core_outs: list of [NT, 64] arrays in (t-major per batch? no: bt = t*8+b)."""
    full = np.zeros((B, S, D_OUT), np.float32)
    for c, o in enumerate(core_outs):
        o = o[:s_steps * BL].reshape(s_steps, BL, D_OUT)  # [t, b, :]
        full[c * BL:(c + 1) * BL, :s_steps] = o.transpose(1, 0, 2)
    return full


# ----------------------------------------------------------------------------
# Harness entry point: kernel(**inputs) -> [B, S, D_OUT] float32
# ----------------------------------------------------------------------------
_CACHE = {}


def _get_nc():
    if "nc" not in _CACHE:
        _CACHE["nc"] = build_kernel(s_steps=S)
    return _CACHE["nc"]


def kernel(x, emb, w_ih, w_hh, b_ih, b_hh, fc_w, fc_b):
    x = np.asarray(x)
    emb = np.asarray(emb, np.float32)
    w_ih = np.asarray(w_ih, np.float32)
    w_hh = np.asarray(w_hh, np.float32)
    b_ih = np.asarray(b_ih, np.float32)
    b_hh = np.asarray(b_hh, np.float32)
    fc_w = np.asarray(fc_w, np.float32)
    fc_b = np.asarray(fc_b, np.float32)

    from concourse.bass_utils import run_bass_kernel_spmd

    per_core = host_prep(x, emb, w_ih, w_hh, b_ih, b_hh, fc_w, fc_b)
    nc = _get_nc()
    res = run_bass_kernel_spmd(
        nc, per_core, core_ids=list(range(NCORES)), trace=False
    )
    return assemble_output([r["out"] for r in res.results])



# revision 4
# speedup vs baseline: 3.3853x; 3.3853x over previous
"""Bass/Tile GRU kernel for trn2, data-parallel over batch on 8 cores.

Model: xe = emb[x]; gi = xe @ w_ih.T + b_ih (+ b_hh for r,z); per step:
  gh = h @ w_hh.T
  r = sig(gi_r + gh_r); z = sig(gi_z + gh_z)
  n = tanh(gi_n + r * (gh_n + bhh_n)); h = (1-z)*n + z*h
then logits = outs @ fc_w.T + fc_b; out = log_softmax(logits)

Cost-model-driven layout (matmul cost = out free size; stationary side free):
all recurrence tensors live transposed, partition = hidden/gate-dim-in-tile,
free = (tile index k or m, batch b).  Per-core B_loc = 8.

  h      [128, 64]   h[j, b] at partition j%128, free (j//128)*8 + b
  psum   [128, 192]  ghT[g*1024+j, b] at partition j%128, free m*8+b,
                     m = (g*1024+j)//128  (m 0..7 r, 8..15 z, 16..23 n)
  gi_sb  [128, 192]  same layout as psum (from phase 1, via DRAM)

Recurrence matmuls are weights-stationary: lhsT = w_hh.T tile [128, 128]
(m,k), rhs = h slice [128, 8] -> 192 matmuls of out-free 8 per step, plus
two identity-lhsT fold matmuls that accumulate gi (r,z) and bhh_n into
psum.  h_new is produced directly in the h layout - no transposes.
"""

import numpy as np
from contextlib import ExitStack

import concourse.bass as bass
import concourse.tile as tile
from concourse import bacc, mybir

F32 = mybir.dt.float32
BF16 = mybir.dt.bfloat16
I16 = mybir.dt.int16
AF = mybir.ActivationFunctionType

VOCAB, D_IN, D_H, D_OUT, B, S = 32000, 512, 1024, 64, 64, 256
P = 128
NCORES = 8
BL = B // NCORES          # 8 batch rows per core
NT = BL * S               # 2048 (b,t) rows per core
FLUSH = 32                # steps between h flushes to DRAM
NM = 3 * D_H // P         # 24 gate-row tiles
KH = D_H // P             # 8 hidden k-tiles
KI = D_IN // P            # 4 input k-tiles
RC = 256                  # phase-1 row-chunk (32 steps * 8 batch)
NCH = NT // RC            # 8 phase-1 chunks


def host_prep(x, emb, w_ih, w_hh, b_ih, b_hh, fc_w, fc_b):
    """Produce the per-core and shared input arrays for the bass kernel."""
    import ml_dtypes
    emb_bf = emb.astype(ml_dtypes.bfloat16)
    # lhsT layouts: [p, k, m, mj]  ->  w[128*m+mj, 128*k+p]
    wih_l = np.ascontiguousarray(
        w_ih.T.reshape(KI, P, NM, P).transpose(1, 0, 2, 3)
    ).astype(ml_dtypes.bfloat16)
    whh_l = np.ascontiguousarray(
        w_hh.T.reshape(KH, P, NM, P).transpose(1, 0, 2, 3)
    ).astype(ml_dtypes.bfloat16)
    # bias for gi: b_ih everywhere, plus b_hh for the r,z gates (m < 16)
    bgiT = b_ih.reshape(NM, P).T.copy()
    bgiT[:, :16] += b_hh.reshape(NM, P).T[:, :16]
    bgiT = bgiT.astype(np.float32)  # [128, 24]
    # bhh_n expanded over batch: [p, 8k+b] = b_hh[2H + 128k + p]
    bhnE = np.repeat(
        b_hh[2 * D_H:].reshape(KH, P).T[:, :, None], BL, axis=2
    ).reshape(P, KH * BL).astype(ml_dtypes.bfloat16)  # [128, 64]
    fcw_l = np.ascontiguousarray(
        fc_w.T.reshape(KH, P, D_OUT)).astype(ml_dtypes.bfloat16)
    fcb = fc_b.astype(np.float32).reshape(D_OUT, 1)
    iden_bf = np.eye(P, dtype=np.float32).astype(ml_dtypes.bfloat16)
    iden32 = np.eye(P, dtype=np.float32)

    shared = dict(
        emb=emb_bf, wih=wih_l, whh=whh_l, bgi=bgiT, bhn=bhnE,
        fcw=fcw_l, fcb=fcb, iden=iden_bf, iden32=iden32,
    )
    per_core = []
    for c in range(NCORES):
        ids = np.ascontiguousarray(
            np.asarray(x[c * BL:(c + 1) * BL, :S]).T).reshape(-1).astype(np.int16)
        tmp = np.zeros((16, P), np.int16)
        i = np.arange(NT)
        li = i % 512
        tmp[li % 16, (i // 512) * 32 + li // 16] = ids
        idx = np.tile(tmp, (8, 1))  # replicated for the 8 Q7 cores
        per_core.append({"idx": idx, **shared})
    return per_core


def build_kernel():
    nc = bacc.Bacc("TRN2", debug=False, num_devices=1)

    idx = nc.dram_tensor("idx", [P, P], I16, kind="ExternalInput").ap()
    emb = nc.dram_tensor("emb", [VOCAB, D_IN], BF16, kind="ExternalInput").ap()
    wih = nc.dram_tensor("wih", [P, KI, NM, P], BF16, kind="ExternalInput").ap()
    whh = nc.dram_tensor("whh", [P, KH, NM, P], BF16, kind="ExternalInput").ap()
    bgi = nc.dram_tensor("bgi", [P, NM], F32, kind="ExternalInput").ap()
    bhn = nc.dram_tensor("bhn", [P, KH * BL], BF16, kind="ExternalInput").ap()
    fcw = nc.dram_tensor("fcw", [KH, P, D_OUT], BF16, kind="ExternalInput").ap()
    fcb = nc.dram_tensor("fcb", [D_OUT, 1], F32, kind="ExternalInput").ap()
    iden = nc.dram_tensor("iden", [P, P], BF16, kind="ExternalInput").ap()
    iden32 = nc.dram_tensor("iden32", [P, P], F32, kind="ExternalInput").ap()
    out = nc.dram_tensor("out", [NT, D_OUT], F32, kind="ExternalOutput").ap()

    with tile.TileContext(nc) as tc, ExitStack() as ctx:
        singles = ctx.enter_context(tc.tile_pool(name="singles", bufs=1))
        dram = ctx.enter_context(tc.tile_pool(name="dram", bufs=1, space="DRAM"))

        # ---- persistent SBUF state ----
        whh_sb = singles.tile([P, KH, NM, P], BF16)
        nc.sync.dma_start(whh_sb[:], whh)
        wih_sb = singles.tile([P, KI, NM, P], BF16)
        nc.sync.dma_start(wih_sb[:], wih)
        bgi_sb = singles.tile([P, NM], F32)
        nc.sync.dma_start(bgi_sb[:], bgi)
        bhn_sb = singles.tile([P, KH * BL], BF16)
        nc.sync.dma_start(bhn_sb[:], bhn)
        fcw_sb = singles.tile([P, KH, D_OUT], BF16)
        nc.sync.dma_start(fcw_sb[:], fcw.rearrange("k p c -> p k c"))
        fcb_sb = singles.tile([D_OUT, 1], F32)
        nc.sync.dma_start(fcb_sb[:], fcb)
        iden_sb = singles.tile([P, P], BF16)
        nc.sync.dma_start(iden_sb[:], iden)
        iden32_sb = singles.tile([P, P], F32)
        nc.sync.dma_start(iden32_sb[:], iden32)
        idx_sb = singles.tile([P, P], I16)
        nc.sync.dma_start(idx_sb[:], idx)

        # gathered embeddings, transposed: xeT[p, gc, kc, i] = xe[512gc+i, 128kc+p]
        xeT = singles.tile([P, NT // 512, KI, 512], BF16)
        for gc in range(NT // 512):
            nc.gpsimd.dma_gather(
                out_ap=xeT[:, gc],
                in_ap=emb,
                idxs_ap=idx_sb[:, gc * 32:(gc + 1) * 32],
                num_idxs=512,
                num_idxs_reg=512,
                elem_size=D_IN,
                transpose=True,
            )

        # DRAM scratch
        gi_dram = dram.tile([S, P, NM * BL], BF16)
        outsT = dram.tile([KH, P, NT], BF16)

        # ---- phase 1: giT = (xe @ w_ih.T + bias) in recurrence layout ----
        p1 = ctx.enter_context(tc.tile_pool(name="p1", bufs=3))
        p1psum = ctx.enter_context(tc.tile_pool(name="p1psum", bufs=2, space="PSUM"))

        def emit_p1_unit(c, m):
            # rows RC*c .. RC*c+RC  (steps 32c..32c+31), gate-row tile m
            ps = p1psum.tile([P, RC], F32, tag="p1ps")
            for k in range(KI):
                nc.tensor.matmul(
                    ps[:],
                    lhsT=wih_sb[:, k, m, :],
                    rhs=xeT[:, c // 2, k, (c % 2) * RC:(c % 2) * RC + RC],
                    start=(k == 0), stop=(k == KI - 1),
                )
            gi_sb = p1.tile([P, RC], BF16, tag="p1gi")
            nc.scalar.activation(gi_sb[:], ps[:], AF.Identity,
                                 bias=bgi_sb[:, m:m + 1])
            gdst = gi_dram[32 * c:32 * c + 32]  # [32, P, 192]
            nc.sync.dma_start(
                bass.AP(tensor=gdst.tensor, offset=gdst.offset + m * BL,
                        ap=[[NM * BL, P], [P * NM * BL, 32], [1, BL]]),
                gi_sb[:],
            )

        # chunk 0 fully before the loop; chunk c emitted during steps of c-1
        for m in range(NM):
            emit_p1_unit(0, m)
        p1_sched = {}  # step -> list of (c, m)
        for c in range(1, NCH):
            for m in range(NM):
                t_emit = 32 * (c - 1) + (m * 32) // NM
                p1_sched.setdefault(t_emit, []).append((c, m))

        # ---- phase 2: recurrence ----
        gi_pool = ctx.enter_context(tc.tile_pool(name="gi", bufs=4))
        gpool = ctx.enter_context(tc.tile_pool(name="gates", bufs=3))
        flushp = ctx.enter_context(tc.tile_pool(name="flush", bufs=2))
        p2ctx = ExitStack()
        psg = p2ctx.enter_context(tc.tile_pool(name="psg", bufs=2, space="PSUM"))

        h0 = singles.tile([P, KH * BL], BF16)
        nc.vector.memset(h0[:], 0.0)
        h_prev = h0
        flush_buf = None

        for t in range(S):
            for (c, m) in p1_sched.get(t, ()):
                emit_p1_unit(c, m)
            tl = t % FLUSH
            if tl == 0:
                flush_buf = flushp.tile([P, FLUSH, KH * BL], BF16)

            gi_sb = gi_pool.tile([P, NM * BL], BF16, tag="gi")
            nc.sync.dma_start(gi_sb[:], gi_dram[t])

            ps = psg.tile([P, NM * BL], F32, tag="ps")
            for k in range(KH):
                rhs = h_prev[:, BL * k:BL * k + BL]
                for m in range(NM):
                    nc.tensor.matmul(
                        ps[:, BL * m:BL * m + BL],
                        lhsT=whh_sb[:, k, m, :],
                        rhs=rhs,
                        start=(k == 0), stop=False,
                        skip_group_check=True,
                    )
            # fold gi (r,z) and bhh_n into psum via identity-lhsT matmuls
            nc.tensor.matmul(ps[:, 0:128], lhsT=iden_sb[:], rhs=gi_sb[:, 0:128],
                             start=False, stop=True, skip_group_check=True)
            nc.tensor.matmul(ps[:, 128:192], lhsT=iden_sb[:], rhs=bhn_sb[:],
                             start=False, stop=True, skip_group_check=True)

            # gates
            r_g = gpool.tile([P, 64], BF16, tag="r")
            nc.scalar.activation(r_g[:], ps[:, 0:64], AF.Sigmoid)
            z_g = gpool.tile([P, 64], BF16, tag="z")
            nc.scalar.activation(z_g[:], ps[:, 64:128], AF.Sigmoid)
            t1 = gpool.tile([P, 64], BF16, tag="t1")
            nc.vector.tensor_mul(t1[:], r_g[:], ps[:, 128:192])
            t2 = gpool.tile([P, 64], BF16, tag="t2")
            nc.vector.tensor_add(t2[:], t1[:], gi_sb[:, 128:192])
            n_g = gpool.tile([P, 64], BF16, tag="n")
            nc.scalar.activation(n_g[:], t2[:], AF.Tanh)
            d_g = gpool.tile([P, 64], BF16, tag="d")
            nc.vector.tensor_sub(d_g[:], h_prev[:], n_g[:])
            t5 = gpool.tile([P, 64], BF16, tag="t5")
            nc.vector.tensor_mul(t5[:], z_g[:], d_g[:])
            h_new = flush_buf[:, tl, :]
            nc.vector.tensor_add(h_new, n_g[:], t5[:])

            if tl == FLUSH - 1:
                t0 = t - tl
                fb = flush_buf.rearrange("p tt (k b) -> p tt k b", k=KH, b=BL)
                for k in range(KH):
                    nc.sync.dma_start(
                        bass.AP(tensor=outsT.tensor,
                                offset=outsT.offset + k * P * NT + BL * t0,
                                ap=[[NT, P], [BL, FLUSH], [1, BL]]),
                        fb[:, :, k, :],
                    )
            h_prev = h_new

        p2ctx.close()

        # ---- phase 3: fc + log_softmax ----
        with tc.tile_pool(name="p3", bufs=3) as p3, \
             tc.tile_pool(name="p3psum", bufs=2, space="PSUM") as p3psum, \
             tc.tile_pool(name="p3psum2", bufs=2, space="PSUM") as p3psum2:
            for q in range(NT // 512):
                oT = p3.tile([P, KH, 512], BF16, tag="oT")
                nc.sync.dma_start(
                    oT[:],
                    outsT.rearrange("k p n -> p k n")[:, :, q * 512:q * 512 + 512],
                )
                ps3 = p3psum.tile([D_OUT, 512], F32)
                for k in range(KH):
                    nc.tensor.matmul(
                        ps3[:], lhsT=fcw_sb[:, k, :], rhs=oT[:, k, :],
                        start=(k == 0), stop=(k == KH - 1),
                    )
                logitsT = p3.tile([D_OUT, 512], F32, tag="logitsT")
                nc.vector.tensor_scalar_add(logitsT[:], ps3[:], fcb_sb[:])
                for w in range(4):
                    ps4 = p3psum2.tile([P, D_OUT], F32)
                    nc.tensor.transpose(
                        ps4[:], logitsT[:, w * 128:w * 128 + 128],
                        iden32_sb[0:D_OUT, 0:D_OUT],
                    )
                    lg = p3.tile([P, D_OUT], F32, tag="lg")
                    nc.vector.tensor_copy(lg[:], ps4[:])
                    mx = p3.tile([P, 1], F32, tag="mx")
                    nc.vector.reduce_max(mx[:], lg[:], axis=mybir.AxisListType.X)
                    nmx = p3.tile([P, 1], F32, tag="nmx")
                    nc.scalar.mul(nmx[:], mx[:], -1.0)
                    ex = p3.tile([P, D_OUT], F32, tag="ex")
                    nc.scalar.activation(ex[:], lg[:], AF.Exp, bias=nmx[:])
                    sm = p3.tile([P, 1], F32, tag="sm")
                    nc.vector.reduce_sum(sm[:], ex[:], axis=mybir.AxisListType.X)
                    lsm = p3.tile([P, 1], F32, tag="lsm")
                    nc.scalar.activation(lsm[:], sm[:], AF.Ln)
                    off = p3.tile([P, 1], F32, tag="off")
                    nc.vector.tensor_add(off[:], mx[:], lsm[:])
                    res = p3.tile([P, D_OUT], F32, tag="res")
                    nc.vector.tensor_scalar_sub(res[:], lg[:], off[:])
                    nc.sync.dma_start(
                        out[q * 512 + w * 128:q * 512 + w * 128 + 128, :], res[:]
                    )

    nc.compile()
    return nc


def assemble_output(core_outs):
    full = np.zeros((B, S, D_OUT), np.float32)
    for c, o in enumerate(core_outs):
        o = o.reshape(S, BL, D_OUT)  # rows are t-major: n = t*8 + b
        full[c * BL:(c + 1) * BL] = o.transpose(1, 0, 2)
    return full


# ----------------------------------------------------------------------------
# Harness entry point: kernel(**inputs) -> [B, S, D_OUT] float32
# ----------------------------------------------------------------------------
_CACHE = {}


def _get_nc():
    if "nc" not in _CACHE:
        _CACHE["nc"] = build_kernel()
    return _CACHE["nc"]


def kernel(x, emb, w_ih, w_hh, b_ih, b_hh, fc_w, fc_b):
    x = np.asarray(x)
    emb = np.asarray(emb, np.float32)
    w_ih = np.asarray(w_ih, np.float32)
    w_hh = np.asarray(w_hh, np.float32)
    b_ih = np.asarray(b_ih, np.float32)
    b_hh = np.asarray(b_hh, np.float32)
    fc_w = np.asarray(fc_w, np.float32)
    fc_b = np.asarray(fc_b, np.float32)

    from concourse.bass_utils import run_bass_kernel_spmd

    per_core = host_prep(x, emb, w_ih, w_hh, b_ih, b_hh, fc_w, fc_b)
    nc = _get_nc()
    res = run_bass_kernel_spmd(
        nc, per_core, core_ids=list(range(NCORES)), trace=False
    )
    return assemble_output([r["out"] for r in res.results])


# revision 14
# speedup vs baseline: 3.8831x; 1.1471x over previous
"""Bass/Tile GRU kernel for trn2, data-parallel over batch on 8 cores.

Model: xe = emb[x]; gi = xe @ w_ih.T + b_ih (+ b_hh for r,z); per step:
  gh = h @ w_hh.T
  r = sig(gi_r + gh_r); z = sig(gi_z + gh_z)
  n = tanh(gi_n + r * (gh_n + bhh_n)); h = (1-z)*n + z*h
then logits = outs @ fc_w.T + fc_b; out = log_softmax(logits)

Cost-model-driven layout (matmul cost = out free size; stationary side free):
all recurrence tensors live transposed, partition = hidden/gate-dim-in-tile,
free = (tile index k or m, batch b).  Per-core B_loc = 8.

  h      [128, 64]   h[j, b] at partition j%128, free (j//128)*8 + b
  psum   [128, 192]  ghT[g*1024+j, b] at partition j%128, free m*8+b,
                     m = (g*1024+j)//128  (m 0..7 r, 8..15 z, 16..23 n)
  gi_sb  [128, 192]  same layout as psum (from phase 1, via DRAM)

Recurrence matmuls are weights-stationary: lhsT = w_hh.T tile [128, 128]
(m,k), rhs = h slice [128, 8] -> 192 matmuls of out-free 8 per step, plus
two identity-lhsT fold matmuls that accumulate gi (r,z) and bhh_n into
psum.  h_new is produced directly in the h layout - no transposes.
"""

import numpy as np
from contextlib import ExitStack

import concourse.bass as bass
import concourse.tile as tile
from concourse import bacc, mybir

F32 = mybir.dt.float32
BF16 = mybir.dt.bfloat16
I16 = mybir.dt.int16
AF = mybir.ActivationFunctionType

VOCAB, D_IN, D_H, D_OUT, B, S = 32000, 512, 1024, 64, 64, 256
P = 128
NCORES = 8
BL = B // NCORES          # 8 batch rows per core
NT = BL * S               # 2048 (b,t) rows per core
FLUSH = 32                # steps between h flushes to DRAM
NM = 3 * D_H // P         # 24 gate-row tiles
KH = D_H // P             # 8 hidden k-tiles
KI = D_IN // P            # 4 input k-tiles
RC = 256                  # phase-1 row-chunk (32 steps * 8 batch)
NCH = NT // RC            # 8 phase-1 chunks


def host_prep(x, emb, w_ih, w_hh, b_ih, b_hh, fc_w, fc_b):
    """Produce the per-core and shared input arrays for the bass kernel."""
    import ml_dtypes
    emb_bf = emb.astype(ml_dtypes.bfloat16)
    # lhsT layouts: [p, k, m, mj]  ->  w[128*m+mj, 128*k+p]
    wih_l = np.ascontiguousarray(
        w_ih.T.reshape(KI, P, NM, P).transpose(1, 0, 2, 3)
    ).astype(ml_dtypes.bfloat16)
    whh_l = np.ascontiguousarray(
        w_hh.T.reshape(KH, P, NM, P).transpose(1, 0, 2, 3)
    ).astype(ml_dtypes.bfloat16)
    # bias for gi: b_ih everywhere, plus b_hh for the r,z gates (m < 16)
    bgiT = b_ih.reshape(NM, P).T.copy()
    bgiT[:, :16] += b_hh.reshape(NM, P).T[:, :16]
    bgiT = bgiT.astype(np.float32)  # [128, 24]
    # bhh_n expanded over batch: [p, 8k+b] = b_hh[2H + 128k + p]
    bhnE = np.repeat(
        b_hh[2 * D_H:].reshape(KH, P).T[:, :, None], BL, axis=2
    ).reshape(P, KH * BL).astype(ml_dtypes.bfloat16)  # [128, 64]
    fcw_l = np.ascontiguousarray(
        fc_w.T.reshape(KH, P, D_OUT)).astype(ml_dtypes.bfloat16)
    fcb = fc_b.astype(np.float32).reshape(D_OUT, 1)
    iden_bf = np.eye(P, dtype=np.float32).astype(ml_dtypes.bfloat16)
    iden32 = np.eye(P, dtype=np.float32)

    shared = dict(
        emb=emb_bf, wih=wih_l, whh=whh_l, bgi=bgiT, bhn=bhnE,
        fcw=fcw_l, fcb=fcb, iden=iden_bf, iden32=iden32,
    )
    per_core = []
    for c in range(NCORES):
        ids = np.ascontiguousarray(
            np.asarray(x[c * BL:(c + 1) * BL, :S]).T).reshape(-1).astype(np.int16)
        tmp = np.zeros((16, P), np.int16)
        i = np.arange(NT)
        li = i % 512
        tmp[li % 16, (i // 512) * 32 + li // 16] = ids
        idx = np.tile(tmp, (8, 1))  # replicated for the 8 Q7 cores
        per_core.append({"idx": idx, **shared})
    return per_core


def build_kernel():
    nc = bacc.Bacc("TRN2", debug=False, num_devices=1)

    idx = nc.dram_tensor("idx", [P, P], I16, kind="ExternalInput").ap()
    emb = nc.dram_tensor("emb", [VOCAB, D_IN], BF16, kind="ExternalInput").ap()
    wih = nc.dram_tensor("wih", [P, KI, NM, P], BF16, kind="ExternalInput").ap()
    whh = nc.dram_tensor("whh", [P, KH, NM, P], BF16, kind="ExternalInput").ap()
    bgi = nc.dram_tensor("bgi", [P, NM], F32, kind="ExternalInput").ap()
    bhn = nc.dram_tensor("bhn", [P, KH * BL], BF16, kind="ExternalInput").ap()
    fcw = nc.dram_tensor("fcw", [KH, P, D_OUT], BF16, kind="ExternalInput").ap()
    fcb = nc.dram_tensor("fcb", [D_OUT, 1], F32, kind="ExternalInput").ap()
    iden = nc.dram_tensor("iden", [P, P], BF16, kind="ExternalInput").ap()
    iden32 = nc.dram_tensor("iden32", [P, P], F32, kind="ExternalInput").ap()
    out = nc.dram_tensor("out", [NT, D_OUT], F32, kind="ExternalOutput").ap()

    with tile.TileContext(nc) as tc, ExitStack() as ctx:
        singles = ctx.enter_context(tc.tile_pool(name="singles", bufs=1))
        dram = ctx.enter_context(tc.tile_pool(name="dram", bufs=1, space="DRAM"))

        # ---- persistent SBUF state ----
        # order: what phase-1 chunk 0 needs comes first (idx, wih, bgi,
        # gathers); whh (6 MB, ~19 us) only gates step 0.
        idx_sb = singles.tile([P, P], I16)
        nc.sync.dma_start(idx_sb[:], idx)
        wih_sb = singles.tile([P, KI, NM, P], BF16)
        nc.sync.dma_start(wih_sb[:], wih)
        bgi_sb = singles.tile([P, NM], F32)
        nc.sync.dma_start(bgi_sb[:], bgi)
        iden_sb = singles.tile([P, P], BF16)
        nc.sync.dma_start(iden_sb[:], iden)

        # gathered embeddings, transposed: xeT[p, gc, kc, i] = xe[512gc+i, 128kc+p]
        xeT = singles.tile([P, NT // 512, KI, 512], BF16)
        for gc in range(NT // 512):
            nc.gpsimd.dma_gather(
                out_ap=xeT[:, gc],
                in_ap=emb,
                idxs_ap=idx_sb[:, gc * 32:(gc + 1) * 32],
                num_idxs=512,
                num_idxs_reg=512,
                elem_size=D_IN,
                transpose=True,
            )

        whh_sb = singles.tile([P, KH, NM, P], BF16)
        nc.sync.dma_start(whh_sb[:], whh)
        bhn_sb = singles.tile([P, KH * BL], BF16)
        nc.sync.dma_start(bhn_sb[:], bhn)
        fcw_sb = singles.tile([P, KH, D_OUT], BF16)
        nc.sync.dma_start(fcw_sb[:], fcw.rearrange("k p c -> p k c"))
        fcb_sb = singles.tile([D_OUT, 1], F32)
        nc.sync.dma_start(fcb_sb[:], fcb)
        iden32_sb = singles.tile([P, P], F32)
        nc.sync.dma_start(iden32_sb[:], iden32)

        # DRAM scratch
        gi_dram = dram.tile([S, P, NM * BL], BF16)
        outsT = dram.tile([KH, P, NT], BF16)

        # ---- phase 1: giT = (xe @ w_ih.T + bias) in recurrence layout ----
        p1 = ctx.enter_context(tc.tile_pool(name="p1", bufs=3))
        p1psum = ctx.enter_context(tc.tile_pool(name="p1psum", bufs=2, space="PSUM"))

        def emit_p1_unit(c, m):
            # rows RC*c .. RC*c+RC  (steps 32c..32c+31), gate-row tile m
            ps = p1psum.tile([P, RC], F32, tag="p1ps")
            for k in range(KI):
                nc.tensor.matmul(
                    ps[:],
                    lhsT=wih_sb[:, k, m, :],
                    rhs=xeT[:, c // 2, k, (c % 2) * RC:(c % 2) * RC + RC],
                    start=(k == 0), stop=(k == KI - 1),
                )
            gi_sb = p1.tile([P, RC], BF16, tag="p1gi")
            nc.vector.tensor_scalar_add(gi_sb[:], ps[:], bgi_sb[:, m:m + 1])
            gdst = gi_dram[32 * c:32 * c + 32]  # [32, P, 192]
            nc.sync.dma_start(
                bass.AP(tensor=gdst.tensor, offset=gdst.offset + m * BL,
                        ap=[[NM * BL, P], [P * NM * BL, 32], [1, BL]]),
                gi_sb[:],
            )

        # chunk 0 fully before the loop; chunk c emitted during steps of c-1
        for m in range(NM):
            emit_p1_unit(0, m)
        p1_sched = {}  # step -> list of (c, m)
        for c in range(1, NCH):
            for m in range(NM):
                t_emit = 32 * (c - 1) + (m * 32) // NM
                p1_sched.setdefault(t_emit, []).append((c, m))

        # ---- phase 2: recurrence ----
        gi_pool = ctx.enter_context(tc.tile_pool(name="gi", bufs=4))
        gpool = ctx.enter_context(tc.tile_pool(name="gates", bufs=3))
        flushp = ctx.enter_context(tc.tile_pool(name="flush", bufs=2))
        p2ctx = ExitStack()
        psg = p2ctx.enter_context(tc.tile_pool(name="psg", bufs=1, space="PSUM"))
        gpsum = p2ctx.enter_context(tc.tile_pool(name="gpsum", bufs=1, space="PSUM"))

        h0 = singles.tile([P, KH * BL], BF16)
        nc.vector.memset(h0[:], 0.0)
        h_prev = h0
        flush_buf = None

        for t in range(S):
            for (c, m) in p1_sched.get(t, ()):
                emit_p1_unit(c, m)
            tl = t % FLUSH
            if tl == 0:
                flush_buf = flushp.tile([P, FLUSH, KH * BL], BF16)

            gi_sb = gi_pool.tile([P, NM * BL], BF16, tag="gi")
            nc.sync.dma_start(gi_sb[:], gi_dram[t])

            # separate psum tiles per gate; burst ordered so ps_r lands
            # first (starts the ACT chain), then ps_n (t1), then ps_z.
            ps_r = psg.tile([P, 64], F32, tag="ps_r")
            ps_z = psg.tile([P, 64], F32, tag="ps_z")
            ps_n = psg.tile([P, 64], F32, tag="ps_n")

            # psum bank zeroing is lazy at whole-bank granularity on a
            # start=True, so the fold must come FIRST (start=True over the
            # whole tile); all gh matmuls then accumulate with start=False.
            def mm_group(pst, m0, fold_rhs):
                nc.tensor.matmul(pst[:], lhsT=iden_sb[:], rhs=fold_rhs,
                                 start=True, stop=False, skip_group_check=True)
                for m in range(m0, m0 + 8):
                    rel = BL * (m - m0)
                    for k in range(KH):
                        nc.tensor.matmul(
                            pst[:, rel:rel + BL],
                            lhsT=whh_sb[:, k, m, :],
                            rhs=h_prev[:, BL * k:BL * k + BL],
                            start=False, stop=(m == m0 + 7 and k == KH - 1),
                            skip_group_check=True,
                        )

            mm_group(ps_r, 0, gi_sb[:, 0:64])
            mm_group(ps_n, 16, bhn_sb[:])
            mm_group(ps_z, 8, gi_sb[:, 64:128])

            # gates: ACT order r, z, tanh; critical chain
            # r -> t1(Pool) -> t2 -> tanh -> v -> h'
            r_g = gpool.tile([P, 64], BF16, tag="rg")
            nc.scalar.activation(r_g[:], ps_r[:], AF.Sigmoid)
            z_g = gpsum.tile([P, 64], F32, tag="zg")
            nc.scalar.activation(z_g[:], ps_z[:], AF.Sigmoid)
            t1 = gpool.tile([P, 64], BF16, tag="t1")
            nc.vector.tensor_mul(t1[:], r_g[:], ps_n[:])
            t2 = gpool.tile([P, 64], BF16, tag="t2")
            nc.vector.tensor_add(t2[:], t1[:], gi_sb[:, 128:192])
            n_g = gpool.tile([P, 64], BF16, tag="n")
            nc.scalar.activation(n_g[:], t2[:], AF.Tanh)
            zh = gpool.tile([P, 64], BF16, tag="zh")
            nc.vector.tensor_mul(zh[:], z_g[:], h_prev[:])
            omz = gpool.tile([P, 64], BF16, tag="omz")
            nc.vector.tensor_scalar(omz[:], z_g[:], -1.0, 1.0,
                                    op0=mybir.AluOpType.mult,
                                    op1=mybir.AluOpType.add)
            v_g = gpool.tile([P, 64], BF16, tag="v")
            nc.vector.tensor_mul(v_g[:], omz[:], n_g[:])
            h_new = flush_buf[:, tl, :]
            nc.vector.tensor_add(h_new, v_g[:], zh[:])

            if tl == FLUSH - 1:
                t0 = t - tl
                fb = flush_buf.rearrange("p tt (k b) -> p tt k b", k=KH, b=BL)
                for k in range(KH):
                    nc.sync.dma_start(
                        bass.AP(tensor=outsT.tensor,
                                offset=outsT.offset + k * P * NT + BL * t0,
                                ap=[[NT, P], [BL, FLUSH], [1, BL]]),
                        fb[:, :, k, :],
                    )
            h_prev = h_new

        p2ctx.close()

        # ---- phase 3: fc + log_softmax ----
        with tc.tile_pool(name="p3", bufs=3) as p3, \
             tc.tile_pool(name="p3psum", bufs=2, space="PSUM") as p3psum, \
             tc.tile_pool(name="p3psum2", bufs=2, space="PSUM") as p3psum2:
            for q in range(NT // 512):
                oT = p3.tile([P, KH, 512], BF16, tag="oT")
                nc.sync.dma_start(
                    oT[:],
                    outsT.rearrange("k p n -> p k n")[:, :, q * 512:q * 512 + 512],
                )
                ps3 = p3psum.tile([D_OUT, 512], F32)
                for k in range(KH):
                    nc.tensor.matmul(
                        ps3[:], lhsT=fcw_sb[:, k, :], rhs=oT[:, k, :],
                        start=(k == 0), stop=(k == KH - 1),
                    )
                logitsT = p3.tile([D_OUT, 512], F32, tag="logitsT")
                nc.vector.tensor_scalar_add(logitsT[:], ps3[:], fcb_sb[:])
                for w in range(4):
                    ps4 = p3psum2.tile([P, D_OUT], F32)
                    nc.tensor.transpose(
                        ps4[:], logitsT[:, w * 128:w * 128 + 128],
                        iden32_sb[0:D_OUT, 0:D_OUT],
                    )
                    lg = p3.tile([P, D_OUT], F32, tag="lg")
                    nc.vector.tensor_copy(lg[:], ps4[:])
                    mx = p3.tile([P, 1], F32, tag="mx")
                    nc.vector.reduce_max(mx[:], lg[:], axis=mybir.AxisListType.X)
                    nmx = p3.tile([P, 1], F32, tag="nmx")
                    nc.scalar.mul(nmx[:], mx[:], -1.0)
                    ex = p3.tile([P, D_OUT], F32, tag="ex")
                    nc.scalar.activation(ex[:], lg[:], AF.Exp, bias=nmx[:])
                    sm = p3.tile([P, 1], F32, tag="sm")
                    nc.vector.reduce_sum(sm[:], ex[:], axis=mybir.AxisListType.X)
                    lsm = p3.tile([P, 1], F32, tag="lsm")
                    nc.scalar.activation(lsm[:], sm[:], AF.Ln)
                    off = p3.tile([P, 1], F32, tag="off")
                    nc.vector.tensor_add(off[:], mx[:], lsm[:])
                    res = p3.tile([P, D_OUT], F32, tag="res")
                    nc.vector.tensor_scalar_sub(res[:], lg[:], off[:])
                    nc.sync.dma_start(
                        out[q * 512 + w * 128:q * 512 + w * 128 + 128, :], res[:]
                    )

    nc.compile()
    return nc


def assemble_output(core_outs):
    full = np.zeros((B, S, D_OUT), np.float32)
    for c, o in enumerate(core_outs):
        o = o.reshape(S, BL, D_OUT)  # rows are t-major: n = t*8 + b
        full[c * BL:(c + 1) * BL] = o.transpose(1, 0, 2)
    return full


# ----------------------------------------------------------------------------
# Harness entry point: kernel(**inputs) -> [B, S, D_OUT] float32
# ----------------------------------------------------------------------------
_CACHE = {}


def _get_nc():
    if "nc" not in _CACHE:
        _CACHE["nc"] = build_kernel()
    return _CACHE["nc"]


def kernel(x, emb, w_ih, w_hh, b_ih, b_hh, fc_w, fc_b):
    x = np.asarray(x)
    emb = np.asarray(emb, np.float32)
    w_ih = np.asarray(w_ih, np.float32)
    w_hh = np.asarray(w_hh, np.float32)
    b_ih = np.asarray(b_ih, np.float32)
    b_hh = np.asarray(b_hh, np.float32)
    fc_w = np.asarray(fc_w, np.float32)
    fc_b = np.asarray(fc_b, np.float32)

    from concourse.bass_utils import run_bass_kernel_spmd

    per_core = host_prep(x, emb, w_ih, w_hh, b_ih, b_hh, fc_w, fc_b)
    nc = _get_nc()
    res = run_bass_kernel_spmd(
        nc, per_core, core_ids=list(range(NCORES)), trace=False
    )
    return assemble_output([r["out"] for r in res.results])


# revision 18
# speedup vs baseline: 3.9148x; 1.0081x over previous
"""Bass/Tile GRU kernel for trn2, data-parallel over batch on 8 cores.

Model: xe = emb[x]; gi = xe @ w_ih.T + b_ih (+ b_hh for r,z); per step:
  gh = h @ w_hh.T
  r = sig(gi_r + gh_r); z = sig(gi_z + gh_z)
  n = tanh(gi_n + r * (gh_n + bhh_n)); h = (1-z)*n + z*h
then logits = outs @ fc_w.T + fc_b; out = log_softmax(logits)

Cost-model-driven layout (matmul cost = out free size; stationary side free):
all recurrence tensors live transposed, partition = hidden/gate-dim-in-tile,
free = (tile index k or m, batch b).  Per-core B_loc = 8.

  h      [128, 64]   h[j, b] at partition j%128, free (j//128)*8 + b
  psum   [128, 192]  ghT[g*1024+j, b] at partition j%128, free m*8+b,
                     m = (g*1024+j)//128  (m 0..7 r, 8..15 z, 16..23 n)
  gi_sb  [128, 192]  same layout as psum (from phase 1, via DRAM)

Recurrence matmuls are weights-stationary: lhsT = w_hh.T tile [128, 128]
(m,k), rhs = h slice [128, 8] -> 192 matmuls of out-free 8 per step, plus
two identity-lhsT fold matmuls that accumulate gi (r,z) and bhh_n into
psum.  h_new is produced directly in the h layout - no transposes.
"""

import numpy as np
from contextlib import ExitStack

import concourse.bass as bass
import concourse.tile as tile
from concourse import bacc, mybir

F32 = mybir.dt.float32
BF16 = mybir.dt.bfloat16
I16 = mybir.dt.int16
AF = mybir.ActivationFunctionType

VOCAB, D_IN, D_H, D_OUT, B, S = 32000, 512, 1024, 64, 64, 256
P = 128
NCORES = 8
BL = B // NCORES          # 8 batch rows per core
NT = BL * S               # 2048 (b,t) rows per core
FLUSH = 32                # steps between h flushes to DRAM
NM = 3 * D_H // P         # 24 gate-row tiles
KH = D_H // P             # 8 hidden k-tiles
KI = D_IN // P            # 4 input k-tiles
RC = 256                  # phase-1 row-chunk (32 steps * 8 batch)
NCH = NT // RC            # 8 phase-1 chunks


def host_prep(x, emb, w_ih, w_hh, b_ih, b_hh, fc_w, fc_b):
    """Produce the per-core and shared input arrays for the bass kernel."""
    import ml_dtypes
    emb_bf = emb.astype(ml_dtypes.bfloat16)
    # lhsT layouts: [p, k, m, mj]  ->  w[128*m+mj, 128*k+p]
    wih_l = np.ascontiguousarray(
        w_ih.T.reshape(KI, P, NM, P).transpose(1, 0, 2, 3)
    ).astype(ml_dtypes.bfloat16)
    whh_l = np.ascontiguousarray(
        w_hh.T.reshape(KH, P, NM, P).transpose(1, 0, 2, 3)
    ).astype(ml_dtypes.bfloat16)
    # bias for gi: b_ih everywhere, plus b_hh for the r,z gates (m < 16)
    bgiT = b_ih.reshape(NM, P).T.copy()
    bgiT[:, :16] += b_hh.reshape(NM, P).T[:, :16]
    bgiT = bgiT.astype(np.float32)  # [128, 24]
    # bhh_n expanded over batch: [p, 8k+b] = b_hh[2H + 128k + p]
    bhnE = np.repeat(
        b_hh[2 * D_H:].reshape(KH, P).T[:, :, None], BL, axis=2
    ).reshape(P, KH * BL).astype(ml_dtypes.bfloat16)  # [128, 64]
    fcw_l = np.ascontiguousarray(
        fc_w.T.reshape(KH, P, D_OUT)).astype(ml_dtypes.bfloat16)
    fcb = fc_b.astype(np.float32).reshape(D_OUT, 1)
    iden_bf = np.eye(P, dtype=np.float32).astype(ml_dtypes.bfloat16)
    iden32 = np.eye(P, dtype=np.float32)

    shared = dict(
        emb=emb_bf, wih=wih_l, whh=whh_l, bgi=bgiT, bhn=bhnE,
        fcw=fcw_l, fcb=fcb, iden=iden_bf, iden32=iden32,
    )
    per_core = []
    for c in range(NCORES):
        ids = np.ascontiguousarray(
            np.asarray(x[c * BL:(c + 1) * BL, :S]).T).reshape(-1).astype(np.int16)
        tmp = np.zeros((16, P), np.int16)
        i = np.arange(NT)
        li = i % 512
        tmp[li % 16, (i // 512) * 32 + li // 16] = ids
        idx = np.tile(tmp, (8, 1))  # replicated for the 8 Q7 cores
        per_core.append({"idx": idx, **shared})
    return per_core


def build_kernel():
    nc = bacc.Bacc("TRN2", debug=False, num_devices=1)

    idx = nc.dram_tensor("idx", [P, P], I16, kind="ExternalInput").ap()
    emb = nc.dram_tensor("emb", [VOCAB, D_IN], BF16, kind="ExternalInput").ap()
    wih = nc.dram_tensor("wih", [P, KI, NM, P], BF16, kind="ExternalInput").ap()
    whh = nc.dram_tensor("whh", [P, KH, NM, P], BF16, kind="ExternalInput").ap()
    bgi = nc.dram_tensor("bgi", [P, NM], F32, kind="ExternalInput").ap()
    bhn = nc.dram_tensor("bhn", [P, KH * BL], BF16, kind="ExternalInput").ap()
    fcw = nc.dram_tensor("fcw", [KH, P, D_OUT], BF16, kind="ExternalInput").ap()
    fcb = nc.dram_tensor("fcb", [D_OUT, 1], F32, kind="ExternalInput").ap()
    iden = nc.dram_tensor("iden", [P, P], BF16, kind="ExternalInput").ap()
    iden32 = nc.dram_tensor("iden32", [P, P], F32, kind="ExternalInput").ap()
    out = nc.dram_tensor("out", [NT, D_OUT], F32, kind="ExternalOutput").ap()

    with tile.TileContext(nc) as tc, ExitStack() as ctx:
        singles = ctx.enter_context(tc.tile_pool(name="singles", bufs=1))
        dram = ctx.enter_context(tc.tile_pool(name="dram", bufs=1, space="DRAM"))

        # ---- persistent SBUF state ----
        # order: what phase-1 chunk 0 needs comes first (idx, wih, bgi,
        # gathers); whh (6 MB, ~19 us) only gates step 0.
        idx_sb = singles.tile([P, P], I16)
        nc.sync.dma_start(idx_sb[:], idx)
        wih_sb = singles.tile([P, KI, NM, P], BF16)
        nc.sync.dma_start(wih_sb[:], wih)
        bgi_sb = singles.tile([P, NM], F32)
        nc.sync.dma_start(bgi_sb[:], bgi)
        iden_sb = singles.tile([P, P], BF16)
        nc.sync.dma_start(iden_sb[:], iden)

        # gathered embeddings, transposed: xeT[p, gc, kc, i] = xe[512gc+i, 128kc+p]
        xeT = singles.tile([P, NT // 512, KI, 512], BF16)
        for gc in range(NT // 512):
            nc.gpsimd.dma_gather(
                out_ap=xeT[:, gc],
                in_ap=emb,
                idxs_ap=idx_sb[:, gc * 32:(gc + 1) * 32],
                num_idxs=512,
                num_idxs_reg=512,
                elem_size=D_IN,
                transpose=True,
            )

        whh_sb = singles.tile([P, KH, NM, P], BF16)
        nc.sync.dma_start(whh_sb[:], whh)
        bhn_sb = singles.tile([P, KH * BL], BF16)
        nc.sync.dma_start(bhn_sb[:], bhn)
        fcw_sb = singles.tile([P, KH, D_OUT], BF16)
        nc.sync.dma_start(fcw_sb[:], fcw.rearrange("k p c -> p k c"))
        fcb_sb = singles.tile([D_OUT, 1], F32)
        nc.sync.dma_start(fcb_sb[:], fcb)
        iden32_sb = singles.tile([P, P], F32)
        nc.sync.dma_start(iden32_sb[:], iden32)

        # DRAM scratch
        gi_dram = dram.tile([S, P, NM * BL], BF16)
        outsT = dram.tile([KH, P, NT], BF16)

        # ---- phase 1: giT = (xe @ w_ih.T + bias) in recurrence layout ----
        p1 = ctx.enter_context(tc.tile_pool(name="p1", bufs=8))
        p1psum = ctx.enter_context(tc.tile_pool(name="p1psum", bufs=2, space="PSUM"))

        def emit_p1_unit(c, m):
            # rows RC*c .. RC*c+RC  (steps 32c..32c+31), gate-row tile m
            ps = p1psum.tile([P, RC], F32, tag="p1ps")
            for k in range(KI):
                nc.tensor.matmul(
                    ps[:],
                    lhsT=wih_sb[:, k, m, :],
                    rhs=xeT[:, c // 2, k, (c % 2) * RC:(c % 2) * RC + RC],
                    start=(k == 0), stop=(k == KI - 1),
                )
            gi_sb = p1.tile([P, RC], BF16, tag="p1gi")
            nc.vector.tensor_scalar_add(gi_sb[:], ps[:], bgi_sb[:, m:m + 1])
            gdst = gi_dram[32 * c:32 * c + 32]  # [32, P, 192]
            nc.sync.dma_start(
                bass.AP(tensor=gdst.tensor, offset=gdst.offset + m * BL,
                        ap=[[NM * BL, P], [P * NM * BL, 32], [1, BL]]),
                gi_sb[:],
            )

        # chunk 0 fully before the loop; chunk c emitted during steps of c-1
        for m in range(NM):
            emit_p1_unit(0, m)
        p1_sched = {}  # step -> list of (c, m)
        for c in range(1, NCH):
            for m in range(NM):
                t_emit = 32 * (c - 1) + (m * 32) // NM
                p1_sched.setdefault(t_emit, []).append((c, m))

        # ---- phase 2: recurrence ----
        gi_pool = ctx.enter_context(tc.tile_pool(name="gi", bufs=6))
        gpool = ctx.enter_context(tc.tile_pool(name="gates", bufs=3))
        flushp = ctx.enter_context(tc.tile_pool(name="flush", bufs=2))
        p2ctx = ExitStack()
        psg = p2ctx.enter_context(tc.tile_pool(name="psg", bufs=1, space="PSUM"))
        gpsum = p2ctx.enter_context(tc.tile_pool(name="gpsum", bufs=1, space="PSUM"))

        h0 = singles.tile([P, KH * BL], BF16)
        nc.vector.memset(h0[:], 0.0)
        h_prev = h0
        flush_buf = None

        for t in range(S):
            tl = t % FLUSH
            if tl == 0:
                flush_buf = flushp.tile([P, FLUSH, KH * BL], BF16)

            gi_sb = gi_pool.tile([P, NM * BL], BF16, tag="gi")
            nc.sync.dma_start(gi_sb[:], gi_dram[t])

            # separate psum tiles per gate; burst ordered so ps_r lands
            # first (starts the ACT chain), then ps_n (t1), then ps_z.
            ps_r = psg.tile([P, 64], F32, tag="ps_r")
            ps_z = psg.tile([P, 64], F32, tag="ps_z")
            ps_n = psg.tile([P, 64], F32, tag="ps_n")

            # psum bank zeroing is lazy at whole-bank granularity on a
            # start=True, so the fold must come FIRST (start=True over the
            # whole tile); all gh matmuls then accumulate with start=False.
            def mm_group(pst, m0, fold_rhs):
                nc.tensor.matmul(pst[:], lhsT=iden_sb[:], rhs=fold_rhs,
                                 start=True, stop=False, skip_group_check=True)
                for m in range(m0, m0 + 8):
                    rel = BL * (m - m0)
                    for k in range(KH):
                        nc.tensor.matmul(
                            pst[:, rel:rel + BL],
                            lhsT=whh_sb[:, k, m, :],
                            rhs=h_prev[:, BL * k:BL * k + BL],
                            start=False, stop=(m == m0 + 7 and k == KH - 1),
                            skip_group_check=True,
                        )

            mm_group(ps_r, 0, gi_sb[:, 0:64])
            mm_group(ps_n, 16, bhn_sb[:])
            mm_group(ps_z, 8, gi_sb[:, 64:128])

            # gates: ACT order r, z, tanh; critical chain
            # r -> t1 -> t2 -> tanh -> v -> h'  (t1 pure-SBUF via nb copy)
            r_g = gpool.tile([P, 64], BF16, tag="rg")
            nc.scalar.activation(r_g[:], ps_r[:], AF.Sigmoid)
            z_g = gpsum.tile([P, 64], F32, tag="zg")
            nc.scalar.activation(z_g[:], ps_z[:], AF.Sigmoid)
            n_sb = gpool.tile([P, 64], BF16, tag="nsb")
            nc.vector.tensor_copy(n_sb[:], ps_n[:])
            t1 = gpool.tile([P, 64], BF16, tag="t1")
            nc.vector.tensor_mul(t1[:], r_g[:], n_sb[:])
            t2 = gpool.tile([P, 64], BF16, tag="t2")
            nc.vector.tensor_add(t2[:], t1[:], gi_sb[:, 128:192])
            n_g = gpool.tile([P, 64], BF16, tag="n")
            nc.scalar.activation(n_g[:], t2[:], AF.Tanh)
            zh = gpool.tile([P, 64], BF16, tag="zh")
            nc.vector.tensor_mul(zh[:], z_g[:], h_prev[:])
            omz = gpool.tile([P, 64], BF16, tag="omz")
            nc.vector.tensor_scalar(omz[:], z_g[:], -1.0, 1.0,
                                    op0=mybir.AluOpType.mult,
                                    op1=mybir.AluOpType.add)
            v_g = gpool.tile([P, 64], BF16, tag="v")
            nc.vector.tensor_mul(v_g[:], omz[:], n_g[:])
            h_new = flush_buf[:, tl, :]
            nc.vector.tensor_add(h_new, v_g[:], zh[:])

            # phase-1 units last: their DVE/DMA work then sits after this
            # step's critical ops in engine order
            for (c, m) in p1_sched.get(t, ()):
                emit_p1_unit(c, m)

            if tl == FLUSH - 1:
                t0 = t - tl
                fb = flush_buf.rearrange("p tt (k b) -> p tt k b", k=KH, b=BL)
                for k in range(KH):
                    nc.sync.dma_start(
                        bass.AP(tensor=outsT.tensor,
                                offset=outsT.offset + k * P * NT + BL * t0,
                                ap=[[NT, P], [BL, FLUSH], [1, BL]]),
                        fb[:, :, k, :],
                    )
            h_prev = h_new

        p2ctx.close()

        # ---- phase 3: fc + log_softmax ----
        with tc.tile_pool(name="p3", bufs=3) as p3, \
             tc.tile_pool(name="p3psum", bufs=2, space="PSUM") as p3psum, \
             tc.tile_pool(name="p3psum2", bufs=2, space="PSUM") as p3psum2:
            for q in range(NT // 512):
                oT = p3.tile([P, KH, 512], BF16, tag="oT")
                nc.sync.dma_start(
                    oT[:],
                    outsT.rearrange("k p n -> p k n")[:, :, q * 512:q * 512 + 512],
                )
                ps3 = p3psum.tile([D_OUT, 512], F32)
                for k in range(KH):
                    nc.tensor.matmul(
                        ps3[:], lhsT=fcw_sb[:, k, :], rhs=oT[:, k, :],
                        start=(k == 0), stop=(k == KH - 1),
                    )
                logitsT = p3.tile([D_OUT, 512], F32, tag="logitsT")
                nc.vector.tensor_scalar_add(logitsT[:], ps3[:], fcb_sb[:])
                for w in range(4):
                    ps4 = p3psum2.tile([P, D_OUT], F32)
                    nc.tensor.transpose(
                        ps4[:], logitsT[:, w * 128:w * 128 + 128],
                        iden32_sb[0:D_OUT, 0:D_OUT],
                    )
                    lg = p3.tile([P, D_OUT], F32, tag="lg")
                    nc.vector.tensor_copy(lg[:], ps4[:])
                    mx = p3.tile([P, 1], F32, tag="mx")
                    nc.vector.reduce_max(mx[:], lg[:], axis=mybir.AxisListType.X)
                    nmx = p3.tile([P, 1], F32, tag="nmx")
                    nc.scalar.mul(nmx[:], mx[:], -1.0)
                    ex = p3.tile([P, D_OUT], F32, tag="ex")
                    nc.scalar.activation(ex[:], lg[:], AF.Exp, bias=nmx[:])
                    sm = p3.tile([P, 1], F32, tag="sm")
                    nc.vector.reduce_sum(sm[:], ex[:], axis=mybir.AxisListType.X)
                    lsm = p3.tile([P, 1], F32, tag="lsm")
                    nc.scalar.activation(lsm[:], sm[:], AF.Ln)
                    off = p3.tile([P, 1], F32, tag="off")
                    nc.vector.tensor_add(off[:], mx[:], lsm[:])
                    res = p3.tile([P, D_OUT], F32, tag="res")
                    nc.vector.tensor_scalar_sub(res[:], lg[:], off[:])
                    nc.sync.dma_start(
                        out[q * 512 + w * 128:q * 512 + w * 128 + 128, :], res[:]
                    )

    nc.compile()
    return nc


def assemble_output(core_outs):
    full = np.zeros((B, S, D_OUT), np.float32)
    for c, o in enumerate(core_outs):
        o = o.reshape(S, BL, D_OUT)  # rows are t-major: n = t*8 + b
        full[c * BL:(c + 1) * BL] = o.transpose(1, 0, 2)
    return full


# ----------------------------------------------------------------------------
# Harness entry point: kernel(**inputs) -> [B, S, D_OUT] float32
# ----------------------------------------------------------------------------
_CACHE = {}


def _get_nc():
    if "nc" not in _CACHE:
        _CACHE["nc"] = build_kernel()
    return _CACHE["nc"]


def kernel(x, emb, w_ih, w_hh, b_ih, b_hh, fc_w, fc_b):
    x = np.asarray(x)
    emb = np.asarray(emb, np.float32)
    w_ih = np.asarray(w_ih, np.float32)
    w_hh = np.asarray(w_hh, np.float32)
    b_ih = np.asarray(b_ih, np.float32)
    b_hh = np.asarray(b_hh, np.float32)
    fc_w = np.asarray(fc_w, np.float32)
    fc_b = np.asarray(fc_b, np.float32)

    from concourse.bass_utils import run_bass_kernel_spmd

    per_core = host_prep(x, emb, w_ih, w_hh, b_ih, b_hh, fc_w, fc_b)
    nc = _get_nc()
    res = run_bass_kernel_spmd(
        nc, per_core, core_ids=list(range(NCORES)), trace=False
    )
    return assemble_output([r["out"] for r in res.results])
